# revision 1
# baseline (speedup 1.0000x reference)
"""Trainium2 Bass kernel for a NeuralODE (forward-Euler scan over a tiny MLP).

Reference computation (per batch row x of `initial`):
    h0 = x @ Wi + bi                                  # [32]
    h_{t+1} = h_t + dt_t * f(h_t),  t = 0..T-2
    f(h) = tanh(tanh(tanh(h@W0+b0)@W1+b1)@W2+b2) @ W3 + b3
    out[t] = h_t @ Wl + bl                            # [8], t = 0..T-1

Device reformulation (exact in exact arithmetic): track the projected state
    p_t = W0^T h_t   (15-dim)     o_t = Wl^T h_t + bl   (8-dim = the output!)
since h_t only ever enters through W0 (layer 0) and Wl (readout):
    z  = tanh(p + b0); z = tanh(z@W1+b1); z = tanh(z@W2+b2)
    p += dt * (z @ (W3@W0) + b3@W0)
    o += dt * (z @ (W3@Wl) + b3@Wl)
This removes the h->z matmul and the separate trajectory projection pass:
the o-part of the state IS the output trajectory.

Per-core layout (8 cores, batch-sharded 4096 -> 512 each):
  512 batch rows = 4 chunks of 128 (columns of every tile).
  State tile s [128 part, 128 cols]: chunk c occupies partitions 32c..32c+31:
     +0..14 = p, +15..22 = o, +23..31 = zeros (junk kept at 0).
  z0/z1 [128,128]: chunk c valid at partitions 32c..32c+14, rest finite junk
     that block-diagonal weights (zero rows/cols) annihilate.
  z2 [128,128]: valid rows as z0; row 124 is a constant 1 (bias row for G);
     rows 111..127 are never written after init.
  Weights live as 128x128 block-diagonal matrices (host-assembled):
     W1bd/W2bd blocks [15,15] at (32c,32c); Gbd blocks [15,23] at (32c,32c)
     = [W3@W0 | W3@Wl], plus row 124 = [b3@W0 | b3@Wl] per chunk.
  Step: act0 -> mm1 -> act1 -> mm2 -> act2 -> mmG -> update(DVE) where
     update s' = (psum_g * dt_t) + s  (scalar_tensor_tensor, dt from SBUF).
  s_t slides through a 2-deep ring of [128, TBUF*128] SBUF blocks; when a
  block completes, its o-rows DMA to DRAM scratch [32, T*128]; the host
  transposes scratch (c,o;t,n) -> out[c*128+n, t, o].
"""

from contextlib import ExitStack

import numpy as np

B, T = 4096, 1000
INIT_DIM, HID, HH, OUT = 16, 32, 15, 8
NCORES = 8
BSH = B // NCORES          # 512 batch rows per core
NCH = 4                    # chunks per core (128 batch cols each)
NSTREAM = 2                # independent dependency chains per core
TBUF = 40                  # time slots per ring block (40 divides 1000)
ONES_ROW = 124             # z2 constant-one row (chunk 3 junk area)
ACT_HI = 111               # act2 writes partitions [0, ACT_HI)


def build_program(t_total=T, tbuf=TBUF, nstream=NSTREAM, accum=False,
                  repeats=1):
    """Build + compile the per-core Bass program (SPMD: same on all cores).

    `nstream` independent dependency chains, each covering a disjoint
    column-slice of the batch, interleave on the engines to hide the
    per-step cross-engine latency (act->mm->act->... is ~3us serial).

    `accum=True` (requires constant dt folded into gbd on the host): the
    state s lives in a PSUM bank and the G-matmul accumulates onto it
    (start=False), removing the DVE update from the per-step critical
    path. A DVE copy snapshots s into the output ring off-path.
    """
    import concourse.tile as tile
    from concourse import bacc, mybir

    F32 = mybir.dt.float32
    Tanh = mybir.ActivationFunctionType.Tanh

    nc = bacc.Bacc("TRN2", target_bir_lowering=False, debug=False)

    s0 = nc.dram_tensor("s0", [128, 128], F32, kind="ExternalInput")
    w1 = nc.dram_tensor("w1bd", [128, 128], F32, kind="ExternalInput")
    w2 = nc.dram_tensor("w2bd", [128, 128], F32, kind="ExternalInput")
    gm = nc.dram_tensor("gbd", [128, 128], F32, kind="ExternalInput")
    bz = nc.dram_tensor("bz", [128, 4], F32, kind="ExternalInput")
    z2i = nc.dram_tensor("z2init", [128, 128], F32, kind="ExternalInput")
    dts = nc.dram_tensor("dts", [128, t_total - 1], F32, kind="ExternalInput")
    ident = nc.dram_tensor("ident", [128, 128], F32, kind="ExternalInput")
    gmo = nc.dram_tensor("gbdo", [128, 32], F32, kind="ExternalInput")
    selo = nc.dram_tensor("selo", [128, 32], F32, kind="ExternalInput")
    scr = nc.dram_tensor("oscr", [32, t_total * 128], F32, kind="ExternalOutput")

    nb = t_total // tbuf
    assert nb * tbuf == t_total
    assert 128 % nstream == 0
    w = 128 // nstream                      # batch cols per stream

    with tile.TileContext(nc) as tc, ExitStack() as ctx:
        const = ctx.enter_context(tc.tile_pool(name="const", bufs=1))
        rings = [ctx.enter_context(tc.tile_pool(name=f"ring{s}", bufs=2))
                 for s in range(nstream)]
        psum = ctx.enter_context(tc.tile_pool(name="psum", bufs=1, space="PSUM"))

        w1_sb = const.tile([128, 128], F32, tag="w1")
        w2_sb = const.tile([128, 128], F32, tag="w2")
        g_sb = const.tile([128, 128], F32, tag="g")
        bz_sb = const.tile([128, 4], F32, tag="bz")
        dts_sb = const.tile([128, t_total - 1], F32, tag="dts")
        nc.sync.dma_start(w1_sb[:], w1.ap())
        nc.sync.dma_start(w2_sb[:], w2.ap())
        nc.sync.dma_start(g_sb[:], gm.ap())
        nc.sync.dma_start(bz_sb[:], bz.ap())
        nc.sync.dma_start(dts_sb[:], dts.ap())
        if accum:
            id_sb = const.tile([128, 128], F32, tag="ident")
            s0_sb = const.tile([128, 128], F32, tag="s0")
            go_sb = const.tile([128, 32], F32, tag="gbdo")
            selo_sb = const.tile([128, 32], F32, tag="selo")
            nc.sync.dma_start(id_sb[:], ident.ap())
            nc.sync.dma_start(s0_sb[:], s0.ap())
            nc.sync.dma_start(go_sb[:], gmo.ap())
            nc.sync.dma_start(selo_sb[:], selo.ap())

        class Stream:
            pass

        streams = []
        for s in range(nstream):
            st = Stream()
            st.lo = s * w
            st.z0 = const.tile([128, w], F32, tag=f"z0_{s}")
            st.z1 = const.tile([128, w], F32, tag=f"z1_{s}")
            st.z2 = const.tile([128, w], F32, tag=f"z2_{s}")
            st.p1 = psum.tile([128, w], F32, tag=f"p1_{s}")
            st.p2 = psum.tile([128, w], F32, tag=f"p2_{s}")
            st.pg = psum.tile([128, w], F32, tag=f"pg_{s}")
            nc.sync.dma_start(st.z2[:], z2i.ap()[:, st.lo:st.lo + w])
            st.prev = None
            st.blk = None
            if accum:
                # p-state accumulator in PSUM (the critical chain reads only
                # this bank), seeded via identity matmul so the PSUM
                # has_written bits are set by the PE itself
                nc.tensor.matmul(st.pg[:], id_sb[:],
                                 s0_sb[:, st.lo:st.lo + w],
                                 start=True, stop=False, skip_group_check=True)
                # o-state accumulator in its own bank: never read by the
                # chain, so snapshot copies cannot stall the next step
                st.po = psum.tile([32, w], F32, tag=f"po_{s}",
                                  name=f"po_{s}")
                nc.tensor.matmul(st.po[:], selo_sb[:],
                                 s0_sb[:, st.lo:st.lo + w],
                                 start=True, stop=False, skip_group_check=True)
            streams.append(st)

        def step_accum(st, slot, blks):
            """Emit the chain producing state s_{slot}; also emit the
            snapshot copy of s_{slot-1} mid-emission so program order puts
            the next chain's act0 (a co-reader of the accumulator bank)
            ahead of the copy."""
            k1, i1 = divmod(slot - 1, tbuf)
            prev_cur = blks[k1][:, i1 * w:(i1 + 1) * w]
            nc.scalar.activation(st.z0[:], st.pg[:], Tanh, bias=bz_sb[:, 0:1])
            nc.tensor.matmul(st.p1[:], w1_sb[:], st.z0[:],
                             start=True, stop=True)
            # snapshot o_{slot-1} into the output ring (off the critical path)
            nc.vector.tensor_copy(prev_cur, st.po[:])
            nc.scalar.activation(st.z1[:], st.p1[:], Tanh, bias=bz_sb[:, 1:2])
            nc.tensor.matmul(st.p2[:], w2_sb[:], st.z1[:],
                             start=True, stop=True)
            nc.scalar.activation(
                st.z2[0:ACT_HI, :], st.p2[0:ACT_HI, :], Tanh,
                bias=bz_sb[0:ACT_HI, 2:3],
            )
            # p += (dt*G_p)^T z2 and o += (dt*G_o)^T z2, accumulated by the PE
            nc.tensor.matmul(st.pg[:], g_sb[:], st.z2[:],
                             start=False, stop=False, skip_group_check=True)
            nc.tensor.matmul(st.po[:], go_sb[:], st.z2[:],
                             start=False, stop=False, skip_group_check=True)

        def step(st, slot, k, i):
            cur = st.blk[:, i * w:(i + 1) * w]
            if slot == 0:
                nc.sync.dma_start(cur, s0.ap()[:, st.lo:st.lo + w])
                st.prev = cur
                return
            nc.scalar.activation(st.z0[:], st.prev, Tanh, bias=bz_sb[:, 0:1])
            nc.tensor.matmul(st.p1[:], w1_sb[:], st.z0[:], start=True, stop=True)
            nc.scalar.activation(st.z1[:], st.p1[:], Tanh, bias=bz_sb[:, 1:2])
            nc.tensor.matmul(st.p2[:], w2_sb[:], st.z1[:], start=True, stop=True)
            nc.scalar.activation(
                st.z2[0:ACT_HI, :], st.p2[0:ACT_HI, :], Tanh,
                bias=bz_sb[0:ACT_HI, 2:3],
            )
            nc.tensor.matmul(st.pg[:], g_sb[:], st.z2[:], start=True, stop=True)
            nc.vector.scalar_tensor_tensor(
                cur, st.pg[:], dts_sb[:, slot - 1:slot], st.prev,
                mybir.AluOpType.mult, mybir.AluOpType.add,
            )
            st.prev = cur

        def drain(st, blk, k):
            # block k's o-rows -> DRAM scratch
            for c in range(NCH):
                nc.sync.dma_start(
                    scr.ap().rearrange("p (t n) -> p t n", n=128)[
                        c * 8:(c + 1) * 8, k * tbuf:(k + 1) * tbuf,
                        st.lo:st.lo + w],
                    blk[32 * c + 15:32 * c + 23, :].rearrange(
                        "p (t n) -> p t n", n=w),
                )

        def drain_o(st, blk, k):
            # accum mode: blk is already [32=(c,o), tbuf*w], matching scr rows
            nc.sync.dma_start(
                scr.ap().rearrange("p (t n) -> p t n", n=128)[
                    :, k * tbuf:(k + 1) * tbuf, st.lo:st.lo + w],
                blk[:, :].rearrange("p (t n) -> p t n", n=w),
            )

        if accum:
            for s, st in enumerate(streams):
                st.blks = {}

            def get_blk(st, s_idx, k):
                if k not in st.blks:
                    st.blks[k] = rings[s_idx].tile(
                        [32, tbuf * w], F32, tag=f"blk{s_idx}",
                        name=f"blk{s_idx}_{k}")
                return st.blks[k]

            for rep in range(repeats):
                if rep:
                    for st in streams:   # fresh ring tiles each repeat
                        st.blks = {}
                for slot in range(1, t_total):
                    k = slot // tbuf
                    for s_idx, st in enumerate(streams):
                        get_blk(st, s_idx, (slot - 1) // tbuf)
                        get_blk(st, s_idx, k)
                        step_accum(st, slot, st.blks)
                        if slot % tbuf == 0:
                            drain_o(st, st.blks[k - 1], k - 1)
            kl, il = divmod(t_total - 1, tbuf)
            for s_idx, st in enumerate(streams):
                cur = st.blks[kl][:, il * w:(il + 1) * w]
                nc.vector.tensor_copy(cur, st.po[:])
                drain_o(st, st.blks[kl], kl)
        else:
            for k in range(nb):
                for s, st in enumerate(streams):
                    st.blk = rings[s].tile([128, tbuf * w], F32, tag=f"blk{s}")
                for i in range(tbuf):
                    slot = k * tbuf + i
                    for st in streams:
                        step(st, slot, k, i)
                for s, st in enumerate(streams):
                    drain(st, st.blk, k)

    nc.compile()
    return nc


def prep_inputs(times, initial, Wi, bi, Wf0, bf0, Wf1, bf1, Wf2, bf2, Wf3, bf3,
                Wl, bl, t_total=T):
    """Host-side prep. Returns (shared input map, per-core s0 list)."""
    f32 = np.float32
    times = np.asarray(times, f32)
    initial = np.asarray(initial, f32)
    Wi, bi = np.asarray(Wi, f32), np.asarray(bi, f32)
    W0, b0 = np.asarray(Wf0, f32), np.asarray(bf0, f32)
    W1, b1 = np.asarray(Wf1, f32), np.asarray(bf1, f32)
    W2, b2 = np.asarray(Wf2, f32), np.asarray(bf2, f32)
    W3, b3 = np.asarray(Wf3, f32), np.asarray(bf3, f32)
    Wl, bl = np.asarray(Wl, f32), np.asarray(bl, f32)

    # block-diagonal weights
    w1bd = np.zeros((128, 128), f32)
    w2bd = np.zeros((128, 128), f32)
    gbd = np.zeros((128, 128), f32)
    G = np.concatenate([W3 @ W0, W3 @ Wl], axis=1)        # [15, 23]
    gc = np.concatenate([b3 @ W0, b3 @ Wl])               # [23]
    for c in range(NCH):
        r = 32 * c
        w1bd[r:r + HH, r:r + HH] = W1
        w2bd[r:r + HH, r:r + HH] = W2
        gbd[r:r + HH, r:r + HH + 8] = G
        gbd[ONES_ROW, r:r + HH + 8] = gc

    bzm = np.zeros((128, 4), f32)
    for c in range(NCH):
        r = 32 * c
        bzm[r:r + HH, 0] = b0
        bzm[r:r + HH, 1] = b1
        bzm[r:r + HH, 2] = b2

    z2init = np.zeros((128, 128), f32)
    z2init[ONES_ROW, :] = 1.0

    dt = times[1:t_total] - times[:t_total - 1]           # [T-1]
    dts_b = np.broadcast_to(dt, (128, t_total - 1)).copy()
    # constant-dt fast path: fold dt into G so the G-matmuls can accumulate
    # the state update directly in PSUM (no separate DVE update op).  The
    # state splits into a p-accumulator (critical chain) and an
    # o-accumulator (output only, snapshotted off-path).
    accum_ok = bool(np.all(dt == dt[0]))
    gbd_dt = (gbd * dt[0]).astype(f32) if accum_ok else gbd
    gbd_p = gbd_dt.copy()
    gbdo = np.zeros((128, 32), f32)
    selo = np.zeros((128, 32), f32)
    for c in range(NCH):
        r = 32 * c
        gbd_p[:, r + HH:r + HH + 8] = 0.0
        gbdo[:, c * 8:(c + 1) * 8] = gbd_dt[:, r + HH:r + HH + 8]
        for j in range(8):
            selo[r + HH + j, c * 8 + j] = 1.0

    # initial state per core: s0[32c+0..14, n] = p0, s0[32c+15..22, n] = o0
    h0 = initial @ Wi + bi                                # [B, 32]
    p0 = h0 @ W0                                          # [B, 15]
    o0 = h0 @ Wl + bl                                     # [B, 8]
    s0_list = []
    for core in range(NCORES):
        s0c = np.zeros((128, 128), f32)
        for c in range(NCH):
            rows = slice(core * BSH + c * 128, core * BSH + (c + 1) * 128)
            s0c[32 * c:32 * c + HH, :] = p0[rows].T
            s0c[32 * c + HH:32 * c + HH + 8, :] = o0[rows].T
        s0_list.append(s0c)

    shared = {
        "w1bd": w1bd, "w2bd": w2bd, "gbd": gbd, "bz": bzm,
        "z2init": z2init, "dts": dts_b, "ident": np.eye(128, dtype=f32),
        "gbd_accum": gbd_p, "gbdo": gbdo, "selo": selo,
    }
    return shared, s0_list, accum_ok


def unshard(scr_list, t_total=T):
    """scratch [32, T*128] per core -> full output [B, T, OUT]."""
    outs = []
    for scr in scr_list:
        s = scr.reshape(NCH, 8, t_total, 128)             # [c, o, t, n]
        outs.append(np.ascontiguousarray(s.transpose(0, 3, 2, 1))
                    .reshape(BSH, t_total, 8))
    return np.concatenate(outs, axis=0)


_CACHE = {}


def _get_program(t_total=T, tbuf=TBUF, nstream=NSTREAM, accum=False,
                 repeats=1):
    key = (t_total, tbuf, nstream, accum, repeats)
    if key not in _CACHE:
        _CACHE[key] = build_program(t_total, tbuf, nstream, accum, repeats)
    return _CACHE[key]


def kernel(**inputs) -> np.ndarray:
    from concourse.bass_utils import run_bass_kernel_spmd

    shared, s0_list, accum_ok = prep_inputs(**inputs)
    nc = _get_program(accum=accum_ok)
    if accum_ok:
        shared = dict(shared, gbd=shared["gbd_accum"])
    shared.pop("gbd_accum")
    in_maps = [dict(shared, s0=s0_list[core]) for core in range(NCORES)]
    res = run_bass_kernel_spmd(nc, in_maps, core_ids=list(range(NCORES)))
    scr_list = [res.results[core]["oscr"] for core in range(NCORES)]
    return unshard(scr_list)



# revision 2
# speedup vs baseline: 1.1567x; 1.1567x over previous
"""Trainium2 Bass kernel for a NeuralODE (forward-Euler scan over a tiny MLP).

Reference computation (per batch row x of `initial`):
    h0 = x @ Wi + bi                                  # [32]
    h_{t+1} = h_t + dt_t * f(h_t),  t = 0..T-2
    f(h) = tanh(tanh(tanh(h@W0+b0)@W1+b1)@W2+b2) @ W3 + b3
    out[t] = h_t @ Wl + bl                            # [8], t = 0..T-1

Device reformulation (exact in exact arithmetic): track the projected state
    p_t = W0^T h_t   (15-dim)     o_t = Wl^T h_t + bl   (8-dim = the output!)
since h_t only ever enters through W0 (layer 0) and Wl (readout):
    z  = tanh(p + b0); z = tanh(z@W1+b1); z = tanh(z@W2+b2)
    p += dt * (z @ (W3@W0) + b3@W0)
    o += dt * (z @ (W3@Wl) + b3@Wl)
The o-part of the state IS the output trajectory.

The run time is dominated by the serial per-step dependency cycle
    act0 -> mm1 -> act1 -> mm2 -> act2 -> mmG -> act0(next step)
whose latency is almost entirely fixed engine/semaphore latency, so the
layout packs the batch as densely as possible per instruction column:

Per-core layout (8 cores, batch-sharded 4096 -> 512 each):
  512 batch rows = 8 chunks of 64 (columns of every tile).
  Chunk k occupies partition rows 15k..15k+14 (rows 0..119); row 120 is a
  constant-one row (z2 only) feeding the folded b3-biases; rows 121..127
  unused (excluded from every matmul contraction).
  Weights are host-assembled 128x128 block-diagonal matrices at 15-row
  pitch; dt is folded into the G matrices so the state update is pure PSUM
  accumulation by the PE (start=False), keeping the DVE off the cycle.
  p-state pg [128,c] and o-state po [64,c] live in persistent PSUM banks
  seeded via identity matmuls (sets the PSUM has_written bits on the PE).
  p1/p2 share one PSUM bank per stream (WAR covered by the z1 RAW edge).
  2 column-streams of 32 cols interleave to keep both engines fed.
  Each step the DVE snapshots po into a [64, tbuf*c] ring block
  (2-deep); complete blocks DMA to DRAM scratch [64, T*64] = (chunk,o;t,n);
  the host transposes scratch to out[c*64+n, t, o].

A pre-compile pass drops semaphore waits that are trivially satisfied by
same-engine program order; the surviving (cross-engine) wait then attaches
to the consuming instruction itself instead of a standalone EventSemaphore,
which would serialize the SEQ-side decode into the dependency cycle.
"""

from collections import defaultdict
from contextlib import ExitStack

import numpy as np

B, T = 4096, 1000
INIT_DIM, HID, HH, OUT = 16, 32, 15, 8
NCORES = 8
BSH = B // NCORES          # 512 batch rows per core
NCH = 8                    # chunks per core (64 batch cols each)
COLS = BSH // NCH          # 64
PITCH = 15                 # chunk partition pitch
ONES_ROW = 120             # z2 constant-one row
ACT_HI = 120               # activations write partitions [0, 120)
TBUF = 40                  # time slots per ring block (divides 1000)
WIDTHS = (32, 32)          # column split across streams

_SYNC_OK = {
    "InstActivation", "InstMatmult", "InstTensorCopy", "InstMemset",
    "InstEventSemaphore", "InstTensorTensor", "InstTensorScalarPtr",
    "InstLdweights", "InstNoOp", "InstTensorReduce", "InstTensorScalar",
}


def strip_redundant_self_waits(nc):
    """Drop sem waits trivially satisfied by same-engine program order.

    A wait (S >= v) on engine-E instruction X is droppable iff every update
    to S module-wide is a plain `sem-inc` from a synchronous (non-DMA)
    instruction on engine E, and the cumulative update value from
    E-instructions preceding X in the same basic block is >= v.  Dropping
    the redundant wait lets the remaining cross-engine wait attach to X
    itself (TRN2 allows one attached wait), so X pre-decodes and fires as
    soon as the semaphore arrives.
    """
    fn = nc.m.functions[0]
    sem_updaters = defaultdict(list)
    for b in fn.blocks:
        for inst in b.instructions:
            si = inst.sync_info
            if si is not None and si.on_update:
                for u in si.on_update:
                    sem_updaters[u.ant_name].append(
                        (inst.engine, type(inst).__name__, u.update_mode))

    def droppable_sem(name, engine):
        ups = sem_updaters.get(name)
        if not ups:
            return False
        return all(e == engine and t in _SYNC_OK and m == "sem-inc"
                   for (e, t, m) in ups)

    for b in fn.blocks:
        cum = defaultdict(int)
        for inst in b.instructions:
            si = inst.sync_info
            if si is not None and si.on_wait:
                keep = [w for w in si.on_wait if not (
                    w.sync_type == "semaphore"
                    and w.wait_mode == "sem-ge-imm"
                    and droppable_sem(w.ant_name, inst.engine)
                    and cum[(inst.engine, w.ant_name)] >= w.wait_value)]
                if len(keep) != len(si.on_wait):
                    si.on_wait = keep
            if si is not None and si.on_update:
                for u in si.on_update:
                    if u.update_mode == "sem-inc":
                        cum[(inst.engine, u.ant_name)] += u.update_value


def build_program(t_total=T, tbuf=TBUF, widths=WIDTHS):
    import concourse.tile as tile
    from concourse import bacc, mybir

    F32 = mybir.dt.float32
    Tanh = mybir.ActivationFunctionType.Tanh

    nc = bacc.Bacc("TRN2", target_bir_lowering=False, debug=False)

    s0p = nc.dram_tensor("s0p", [128, COLS], F32, kind="ExternalInput")
    s0o = nc.dram_tensor("s0o", [64, COLS], F32, kind="ExternalInput")
    w1 = nc.dram_tensor("w1bd", [128, 128], F32, kind="ExternalInput")
    w2 = nc.dram_tensor("w2bd", [128, 128], F32, kind="ExternalInput")
    gm = nc.dram_tensor("gbd", [128, 128], F32, kind="ExternalInput")
    gmo = nc.dram_tensor("gbdo", [128, 64], F32, kind="ExternalInput")
    bz = nc.dram_tensor("bz", [128, 4], F32, kind="ExternalInput")
    z2i = nc.dram_tensor("z2init", [128, COLS], F32, kind="ExternalInput")
    ident = nc.dram_tensor("ident", [128, 128], F32, kind="ExternalInput")
    scr = nc.dram_tensor("oscr", [64, t_total * COLS], F32,
                         kind="ExternalOutput")

    nb = t_total // tbuf
    assert nb * tbuf == t_total
    assert sum(widths) == COLS
    nstream = len(widths)

    with tile.TileContext(nc) as tc, ExitStack() as ctx:
        const = ctx.enter_context(tc.tile_pool(name="const", bufs=1))
        rings = [ctx.enter_context(tc.tile_pool(name=f"ring{s}", bufs=2))
                 for s in range(nstream)]
        psum = ctx.enter_context(tc.tile_pool(name="psum", bufs=1,
                                              space="PSUM"))

        # startup-critical tensors first so the scan starts while the
        # remaining weights stream in
        id_sb = const.tile([128, 128], F32, tag="ident")
        s0p_sb = const.tile([128, COLS], F32, tag="s0p")
        s0o_sb = const.tile([64, COLS], F32, tag="s0o")
        bz_sb = const.tile([128, 4], F32, tag="bz")
        z2_sb = const.tile([128, COLS], F32, tag="z2")
        w1_sb = const.tile([128, 128], F32, tag="w1")
        w2_sb = const.tile([128, 128], F32, tag="w2")
        g_sb = const.tile([128, 128], F32, tag="g")
        go_sb = const.tile([128, 64], F32, tag="go")
        nc.sync.dma_start(id_sb[:], ident.ap())
        nc.sync.dma_start(s0p_sb[:], s0p.ap())
        nc.sync.dma_start(s0o_sb[:], s0o.ap())
        nc.sync.dma_start(bz_sb[:], bz.ap())
        nc.sync.dma_start(z2_sb[:], z2i.ap())
        nc.sync.dma_start(w1_sb[:], w1.ap())
        nc.sync.dma_start(w2_sb[:], w2.ap())
        nc.sync.dma_start(g_sb[:], gm.ap())
        nc.sync.dma_start(go_sb[:], gmo.ap())

        z0_sb = const.tile([128, COLS], F32, tag="z0")
        z1_sb = const.tile([128, COLS], F32, tag="z1")

        class Stream:
            pass

        streams = []
        for s in range(nstream):
            st = Stream()
            st.lo = sum(widths[:s])
            st.w = widths[s]
            sl = slice(st.lo, st.lo + st.w)
            st.z0 = z0_sb[:, sl]
            st.z1 = z1_sb[:, sl]
            st.z2 = z2_sb[:, sl]
            st.p1 = psum.tile([128, st.w], F32, tag=f"p12_{s}",
                              name=f"p12_{s}")[:]
            st.p2 = st.p1
            st.pg = psum.tile([128, st.w], F32, tag=f"pg_{s}",
                              name=f"pg_{s}")[:]
            st.po = psum.tile([64, st.w], F32, tag=f"po_{s}",
                              name=f"po_{s}")[:]
            # seed accumulators via the PE (sets PSUM has_written bits)
            nc.tensor.matmul(st.pg, id_sb[:], s0p_sb[:, sl],
                             start=True, stop=False, skip_group_check=True)
            nc.tensor.matmul(st.po, id_sb[0:64, 0:64], s0o_sb[:, sl],
                             start=True, stop=False, skip_group_check=True)
            st.blks = {}
            streams.append(st)

        def get_blk(st, s_idx, k):
            if k not in st.blks:
                st.blks[k] = rings[s_idx].tile(
                    [64, tbuf * st.w], F32, tag=f"blk{s_idx}",
                    name=f"blk{s_idx}_{k}")
            return st.blks[k]

        def drain_o(st, blk, k):
            nc.sync.dma_start(
                scr.ap().rearrange("p (t n) -> p t n", n=COLS)[
                    :, k * tbuf:(k + 1) * tbuf, st.lo:st.lo + st.w],
                blk[:, :].rearrange("p (t n) -> p t n", n=st.w),
            )

        K1 = 120   # contraction rows for W1/W2 matmuls
        KG = 121   # contraction rows for G matmuls (incl ones-row)

        for slot in range(1, t_total):
            k = slot // tbuf
            k1, i1 = divmod(slot - 1, tbuf)
            for s_idx, st in enumerate(streams):
                get_blk(st, s_idx, k1)
                get_blk(st, s_idx, k)
            for st in streams:
                nc.scalar.activation(st.z0[0:ACT_HI, :], st.pg[0:ACT_HI, :],
                                     Tanh, bias=bz_sb[0:ACT_HI, 0:1])
            for st in streams:
                nc.tensor.matmul(st.p1, w1_sb[0:K1, :], st.z0[0:K1, :],
                                 start=True, stop=True)
            for st in streams:
                # snapshot o_{slot-1} into the output ring (off the cycle)
                prev = st.blks[k1][:, i1 * st.w:(i1 + 1) * st.w]
                nc.vector.tensor_copy(prev, st.po)
            for st in streams:
                nc.scalar.activation(st.z1[0:ACT_HI, :], st.p1[0:ACT_HI, :],
                                     Tanh, bias=bz_sb[0:ACT_HI, 1:2])
            for st in streams:
                nc.tensor.matmul(st.p2, w2_sb[0:K1, :], st.z1[0:K1, :],
                                 start=True, stop=True)
            for st in streams:
                nc.scalar.activation(st.z2[0:ACT_HI, :], st.p2[0:ACT_HI, :],
                                     Tanh, bias=bz_sb[0:ACT_HI, 2:3])
            for st in streams:
                nc.tensor.matmul(st.pg, g_sb[0:KG, :], st.z2[0:KG, :],
                                 start=False, stop=False,
                                 skip_group_check=True)
            for st in streams:
                nc.tensor.matmul(st.po, go_sb[0:KG, :], st.z2[0:KG, :],
                                 start=False, stop=False,
                                 skip_group_check=True)
            if slot % tbuf == 0:
                for st in streams:
                    drain_o(st, st.blks[k - 1], k - 1)

        kl, il = divmod(t_total - 1, tbuf)
        for st in streams:
            cur = st.blks[kl][:, il * st.w:(il + 1) * st.w]
            nc.vector.tensor_copy(cur, st.po)
        for st in streams:
            drain_o(st, st.blks[kl], kl)

    strip_redundant_self_waits(nc)
    nc.compile()
    return nc


def prep_inputs(times, initial, Wi, bi, Wf0, bf0, Wf1, bf1, Wf2, bf2, Wf3, bf3,
                Wl, bl, t_total=T):
    """Host-side prep. Returns (shared input map, per-core s0p/s0o lists)."""
    f32 = np.float32
    times = np.asarray(times, f32)
    initial = np.asarray(initial, f32)
    Wi, bi = np.asarray(Wi, f32), np.asarray(bi, f32)
    W0, b0 = np.asarray(Wf0, f32), np.asarray(bf0, f32)
    W1, b1 = np.asarray(Wf1, f32), np.asarray(bf1, f32)
    W2, b2 = np.asarray(Wf2, f32), np.asarray(bf2, f32)
    W3, b3 = np.asarray(Wf3, f32), np.asarray(bf3, f32)
    Wl, bl = np.asarray(Wl, f32), np.asarray(bl, f32)

    dt = times[1:t_total] - times[:t_total - 1]
    assert np.all(dt == dt[0]), "kernel requires a constant time step"
    dt0 = float(dt[0])

    w1bd = np.zeros((128, 128), f32)
    w2bd = np.zeros((128, 128), f32)
    gbd = np.zeros((128, 128), f32)
    gobd = np.zeros((128, 64), f32)
    Gp = (W3 @ W0) * dt0                                   # [15, 15]
    Go = (W3 @ Wl) * dt0                                   # [15, 8]
    gcp = (b3 @ W0) * dt0                                  # [15]
    gco = (b3 @ Wl) * dt0                                  # [8]
    for c in range(NCH):
        r = PITCH * c
        w1bd[r:r + HH, r:r + HH] = W1
        w2bd[r:r + HH, r:r + HH] = W2
        gbd[r:r + HH, r:r + HH] = Gp
        gbd[ONES_ROW, r:r + HH] = gcp
        gobd[r:r + HH, 8 * c:8 * c + 8] = Go
        gobd[ONES_ROW, 8 * c:8 * c + 8] = gco

    bzm = np.zeros((128, 4), f32)
    for c in range(NCH):
        r = PITCH * c
        bzm[r:r + HH, 0] = b0
        bzm[r:r + HH, 1] = b1
        bzm[r:r + HH, 2] = b2

    z2init = np.zeros((128, COLS), f32)
    z2init[ONES_ROW, :] = 1.0

    # initial state per core: p0 = h0@W0, o0 = h0@Wl + bl
    h0 = initial @ Wi + bi                                 # [B, 32]
    p0 = h0 @ W0                                           # [B, 15]
    o0 = h0 @ Wl + bl                                      # [B, 8]
    s0p_list, s0o_list = [], []
    for core in range(NCORES):
        sp = np.zeros((128, COLS), f32)
        so = np.zeros((64, COLS), f32)
        for c in range(NCH):
            rows = slice(core * BSH + c * COLS, core * BSH + (c + 1) * COLS)
            sp[PITCH * c:PITCH * c + HH, :] = p0[rows].T
            so[8 * c:8 * c + 8, :] = o0[rows].T
        s0p_list.append(sp)
        s0o_list.append(so)

    shared = {
        "w1bd": w1bd, "w2bd": w2bd, "gbd": gbd, "gbdo": gobd, "bz": bzm,
        "z2init": z2init, "ident": np.eye(128, dtype=f32),
    }
    return shared, s0p_list, s0o_list


def unshard(scr_list, t_total=T):
    """scratch [64, T*64] per core -> full output [B, T, OUT]."""
    outs = []
    for scr in scr_list:
        s = scr.reshape(NCH, 8, t_total, COLS)             # [c, o, t, n]
        outs.append(np.ascontiguousarray(s.transpose(0, 3, 2, 1))
                    .reshape(BSH, t_total, 8))
    return np.concatenate(outs, axis=0)


_CACHE = {}


def _get_program(t_total=T, tbuf=TBUF, widths=WIDTHS):
    key = (t_total, tbuf, widths)
    if key not in _CACHE:
        _CACHE[key] = build_program(t_total, tbuf, widths)
    return _CACHE[key]


def kernel(**inputs) -> np.ndarray:
    from concourse.bass_utils import run_bass_kernel_spmd

    shared, s0p_list, s0o_list = prep_inputs(**inputs)
    nc = _get_program()
    in_maps = [dict(shared, s0p=s0p_list[core], s0o=s0o_list[core])
               for core in range(NCORES)]
    res = run_bass_kernel_spmd(nc, in_maps, core_ids=list(range(NCORES)))
    scr_list = [res.results[core]["oscr"] for core in range(NCORES)]
    return unshard(scr_list)


# revision 3
# speedup vs baseline: 1.1734x; 1.0144x over previous
"""Trainium2 Bass kernel for a NeuralODE (forward-Euler scan over a tiny MLP).

Reference computation (per batch row x of `initial`):
    h0 = x @ Wi + bi                                  # [32]
    h_{t+1} = h_t + dt * f(h_t),  t = 0..T-2
    f(h) = tanh(tanh(tanh(h@W0+b0)@W1+b1)@W2+b2) @ W3 + b3
    out[t] = h_t @ Wl + bl                            # [8], t = 0..T-1

Device reformulation (exact in exact arithmetic): track the projected state
    p_t = W0^T h_t   (15-dim)     o_t = Wl^T h_t + bl   (8-dim = the output!)
since h_t only ever enters through W0 (layer 0) and Wl (readout):
    z  = tanh(p + b0); z = tanh(z@W1+b1); z = tanh(z@W2+b2)
    p += dt * (z @ (W3@W0) + b3@W0)
    o += dt * (z @ (W3@Wl) + b3@Wl)
The o-part of the state IS the output trajectory.

Total time is dominated by the serial per-step dependency cycle
    act0 -> mm1 -> act1 -> mm2 -> act2 -> mmG -> act0(next step)
whose latency is almost entirely fixed engine/semaphore pipeline latency
(ACT ~370ns SBUF access+ack, PE ~173ns, ~120ns sem hops), so the design
minimizes per-instruction column counts and keeps everything else off the
cycle:

Per-core layout (8 cores, batch-sharded 4096 -> 512 each):
  512 batch rows = 8 chunks of 64 (columns of every tile).
  Chunk k occupies partition rows 15k..15k+14 (rows 0..119); row 120 is a
  constant-one row (z2 only) feeding the folded b3-biases; rows 121..127
  unused (excluded from every matmul contraction).
  Weights are host-assembled 128x128 block-diagonal matrices at 15-row
  pitch; dt is folded into the G matrices so the p-update is pure PSUM
  accumulation by the PE (start=False), keeping the DVE off the cycle.
  3 column-streams (22/21/21 cols) interleave so the cycle runs with
  minimal per-instruction processing time while the ACT engine (~93%
  busy) still keeps up.
  PSUM banks (8 available): per stream a persistent p-accumulator pg
  [128,w] (seeded via identity matmul so the PE sets the has_written
  bits) and one bank shared by p1/p2 (the WAR is covered by the z1 RAW
  edge); plus a single shared podelta bank [64,64] that each stream's
  G_o matmul writes as a fresh start/stop group each step.
  The o-trajectory is accumulated OFF the cycle by the DVE:
      blk[slot t] = blk[slot t-1] + podelta_t
  chaining through a [64, tbuf*w] ring (2-deep) whose complete blocks
  DMA to DRAM scratch [64, T*64] = (chunk,o; t,n); the host transposes
  scratch to out[c*64+n, t, o].

A pre-compile pass drops semaphore waits that are trivially satisfied by
same-engine program order; the surviving (cross-engine) wait then attaches
to the consuming instruction itself instead of a standalone EventSemaphore,
which would serialize the SEQ-side decode into the dependency cycle.
"""

from collections import defaultdict
from contextlib import ExitStack

import numpy as np

B, T = 4096, 1000
INIT_DIM, HID, HH, OUT = 16, 32, 15, 8
NCORES = 8
BSH = B // NCORES          # 512 batch rows per core
NCH = 8                    # chunks per core (64 batch cols each)
COLS = BSH // NCH          # 64
PITCH = 15                 # chunk partition pitch
ONES_ROW = 120             # z2 constant-one row
ACT_HI = 120               # activations write partitions [0, 120)
TBUF = 10                  # time slots per ring block (divides 1000)
WIDTHS = (22, 21, 21)      # column split across streams

_SYNC_OK = {
    "InstActivation", "InstMatmult", "InstTensorCopy", "InstMemset",
    "InstEventSemaphore", "InstTensorTensor", "InstTensorScalarPtr",
    "InstLdweights", "InstNoOp", "InstTensorReduce", "InstTensorScalar",
}


def strip_redundant_self_waits(nc):
    """Drop sem waits trivially satisfied by same-engine program order.

    A wait (S >= v) on engine-E instruction X is droppable iff every update
    to S module-wide is a plain `sem-inc` from a synchronous (non-DMA)
    instruction on engine E, and the cumulative update value from
    E-instructions preceding X in the same basic block is >= v.  Dropping
    the redundant wait lets the remaining cross-engine wait attach to X
    itself (TRN2 allows one attached wait per instruction), so X
    pre-decodes and fires as soon as the producer's semaphore arrives.
    """
    fn = nc.m.functions[0]
    sem_updaters = defaultdict(list)
    for b in fn.blocks:
        for inst in b.instructions:
            si = inst.sync_info
            if si is not None and si.on_update:
                for u in si.on_update:
                    sem_updaters[u.ant_name].append(
                        (inst.engine, type(inst).__name__, u.update_mode))

    def droppable_sem(name, engine):
        ups = sem_updaters.get(name)
        if not ups:
            return False
        return all(e == engine and t in _SYNC_OK and m == "sem-inc"
                   for (e, t, m) in ups)

    for b in fn.blocks:
        cum = defaultdict(int)
        for inst in b.instructions:
            si = inst.sync_info
            if si is not None and si.on_wait:
                keep = [w for w in si.on_wait if not (
                    w.sync_type == "semaphore"
                    and w.wait_mode == "sem-ge-imm"
                    and droppable_sem(w.ant_name, inst.engine)
                    and cum[(inst.engine, w.ant_name)] >= w.wait_value)]
                if len(keep) != len(si.on_wait):
                    si.on_wait = keep
            if si is not None and si.on_update:
                for u in si.on_update:
                    if u.update_mode == "sem-inc":
                        cum[(inst.engine, u.ant_name)] += u.update_value


def build_program(t_total=T, tbuf=TBUF, widths=WIDTHS):
    import concourse.tile as tile
    from concourse import bacc, mybir

    F32 = mybir.dt.float32
    Tanh = mybir.ActivationFunctionType.Tanh

    nc = bacc.Bacc("TRN2", target_bir_lowering=False, debug=False)

    s0p = nc.dram_tensor("s0p", [128, COLS], F32, kind="ExternalInput")
    s0o = nc.dram_tensor("s0o", [64, COLS], F32, kind="ExternalInput")
    w1 = nc.dram_tensor("w1bd", [128, 128], F32, kind="ExternalInput")
    w2 = nc.dram_tensor("w2bd", [128, 128], F32, kind="ExternalInput")
    gm = nc.dram_tensor("gbd", [128, 128], F32, kind="ExternalInput")
    gmo = nc.dram_tensor("gbdo", [128, 64], F32, kind="ExternalInput")
    bz = nc.dram_tensor("bz", [128, 4], F32, kind="ExternalInput")
    z2i = nc.dram_tensor("z2init", [128, COLS], F32, kind="ExternalInput")
    ident = nc.dram_tensor("ident", [128, 128], F32, kind="ExternalInput")
    scr = nc.dram_tensor("oscr", [64, t_total * COLS], F32,
                         kind="ExternalOutput")

    nb = t_total // tbuf
    assert nb * tbuf == t_total
    assert sum(widths) == COLS
    nstream = len(widths)

    with tile.TileContext(nc) as tc, ExitStack() as ctx:
        const = ctx.enter_context(tc.tile_pool(name="const", bufs=1))
        rings = [ctx.enter_context(tc.tile_pool(name=f"ring{s}", bufs=2))
                 for s in range(nstream)]
        psum = ctx.enter_context(tc.tile_pool(name="psum", bufs=1,
                                              space="PSUM"))

        # startup-critical tensors first so the scan starts while the
        # remaining weights stream in
        id_sb = const.tile([128, 128], F32, tag="ident")
        s0p_sb = const.tile([128, COLS], F32, tag="s0p")
        s0o_sb = const.tile([64, COLS], F32, tag="s0o")
        bz_sb = const.tile([128, 4], F32, tag="bz")
        z2_sb = const.tile([128, COLS], F32, tag="z2")
        w1_sb = const.tile([128, 128], F32, tag="w1")
        w2_sb = const.tile([128, 128], F32, tag="w2")
        g_sb = const.tile([128, 128], F32, tag="g")
        go_sb = const.tile([128, 64], F32, tag="go")
        nc.sync.dma_start(id_sb[:], ident.ap())
        nc.sync.dma_start(s0p_sb[:], s0p.ap())
        nc.sync.dma_start(s0o_sb[:], s0o.ap())
        nc.sync.dma_start(bz_sb[:], bz.ap())
        nc.sync.dma_start(z2_sb[:], z2i.ap())
        nc.sync.dma_start(w1_sb[:], w1.ap())
        nc.sync.dma_start(w2_sb[:], w2.ap())
        nc.sync.dma_start(g_sb[:], gm.ap())
        nc.sync.dma_start(go_sb[:], gmo.ap())

        z0_sb = const.tile([128, COLS], F32, tag="z0")
        z1_sb = const.tile([128, COLS], F32, tag="z1")
        podelta = psum.tile([64, COLS], F32, tag="podelta")

        class Stream:
            pass

        streams = []
        for s in range(nstream):
            st = Stream()
            st.lo = sum(widths[:s])
            st.w = widths[s]
            sl = slice(st.lo, st.lo + st.w)
            st.z0 = z0_sb[:, sl]
            st.z1 = z1_sb[:, sl]
            st.z2 = z2_sb[:, sl]
            st.p1 = psum.tile([128, st.w], F32, tag=f"p12_{s}",
                              name=f"p12_{s}")[:]
            st.p2 = st.p1
            st.pg = psum.tile([128, st.w], F32, tag=f"pg_{s}",
                              name=f"pg_{s}")[:]
            st.pd = podelta[:, sl]
            # seed the p accumulator via the PE (sets PSUM has_written bits)
            nc.tensor.matmul(st.pg, id_sb[:], s0p_sb[:, sl],
                             start=True, stop=False, skip_group_check=True)
            st.blks = {}
            streams.append(st)

        def get_blk(st, s_idx, k):
            if k not in st.blks:
                st.blks[k] = rings[s_idx].tile(
                    [64, tbuf * st.w], F32, tag=f"blk{s_idx}",
                    name=f"blk{s_idx}_{k}")
            return st.blks[k]

        def drain_o(st, blk, k):
            nc.sync.dma_start(
                scr.ap().rearrange("p (t n) -> p t n", n=COLS)[
                    :, k * tbuf:(k + 1) * tbuf, st.lo:st.lo + st.w],
                blk[:, :].rearrange("p (t n) -> p t n", n=st.w),
            )

        K1 = 120   # contraction rows for W1/W2 matmuls
        KG = 121   # contraction rows for G matmuls (incl ones-row)

        # ring slot 0 <- o_0 (initial readout)
        for s_idx, st in enumerate(streams):
            blk0 = get_blk(st, s_idx, 0)
            nc.vector.tensor_copy(blk0[:, 0:st.w],
                                  s0o_sb[:, st.lo:st.lo + st.w])

        for slot in range(1, t_total):
            k = slot // tbuf
            k1, i1 = divmod(slot - 1, tbuf)
            for s_idx, st in enumerate(streams):
                get_blk(st, s_idx, k1)
                get_blk(st, s_idx, k)
            for st in streams:
                nc.scalar.activation(st.z0[0:ACT_HI, :], st.pg[0:ACT_HI, :],
                                     Tanh, bias=bz_sb[0:ACT_HI, 0:1])
            for st in streams:
                nc.tensor.matmul(st.p1, w1_sb[0:K1, :], st.z0[0:K1, :],
                                 start=True, stop=True)
            for st in streams:
                nc.scalar.activation(st.z1[0:ACT_HI, :], st.p1[0:ACT_HI, :],
                                     Tanh, bias=bz_sb[0:ACT_HI, 1:2])
            for st in streams:
                nc.tensor.matmul(st.p2, w2_sb[0:K1, :], st.z1[0:K1, :],
                                 start=True, stop=True)
            for st in streams:
                nc.scalar.activation(st.z2[0:ACT_HI, :], st.p2[0:ACT_HI, :],
                                     Tanh, bias=bz_sb[0:ACT_HI, 2:3])
            for st in streams:
                nc.tensor.matmul(st.pg, g_sb[0:KG, :], st.z2[0:KG, :],
                                 start=False, stop=False,
                                 skip_group_check=True)
            for st in streams:
                nc.tensor.matmul(st.pd, go_sb[0:KG, :], st.z2[0:KG, :],
                                 start=True, stop=True,
                                 skip_group_check=True)
            # o_t = o_{t-1} + podelta, chained through the ring by the DVE
            ks, isl = divmod(slot, tbuf)
            for st in streams:
                prev = st.blks[k1][:, i1 * st.w:(i1 + 1) * st.w]
                cur = st.blks[ks][:, isl * st.w:(isl + 1) * st.w]
                nc.vector.tensor_add(cur, prev, st.pd)
            if slot % tbuf == 0:
                for st in streams:
                    drain_o(st, st.blks[k - 1], k - 1)

        kl = (t_total - 1) // tbuf
        for st in streams:
            drain_o(st, st.blks[kl], kl)

    strip_redundant_self_waits(nc)
    nc.compile()
    return nc


def prep_inputs(times, initial, Wi, bi, Wf0, bf0, Wf1, bf1, Wf2, bf2, Wf3, bf3,
                Wl, bl, t_total=T):
    """Host-side prep. Returns (shared input map, per-core s0p/s0o lists)."""
    f32 = np.float32
    times = np.asarray(times, f32)
    initial = np.asarray(initial, f32)
    Wi, bi = np.asarray(Wi, f32), np.asarray(bi, f32)
    W0, b0 = np.asarray(Wf0, f32), np.asarray(bf0, f32)
    W1, b1 = np.asarray(Wf1, f32), np.asarray(bf1, f32)
    W2, b2 = np.asarray(Wf2, f32), np.asarray(bf2, f32)
    W3, b3 = np.asarray(Wf3, f32), np.asarray(bf3, f32)
    Wl, bl = np.asarray(Wl, f32), np.asarray(bl, f32)

    dt = times[1:t_total] - times[:t_total - 1]
    assert np.all(dt == dt[0]), "kernel requires a constant time step"
    dt0 = float(dt[0])

    w1bd = np.zeros((128, 128), f32)
    w2bd = np.zeros((128, 128), f32)
    gbd = np.zeros((128, 128), f32)
    gobd = np.zeros((128, 64), f32)
    Gp = (W3 @ W0) * dt0                                   # [15, 15]
    Go = (W3 @ Wl) * dt0                                   # [15, 8]
    gcp = (b3 @ W0) * dt0                                  # [15]
    gco = (b3 @ Wl) * dt0                                  # [8]
    for c in range(NCH):
        r = PITCH * c
        w1bd[r:r + HH, r:r + HH] = W1
        w2bd[r:r + HH, r:r + HH] = W2
        gbd[r:r + HH, r:r + HH] = Gp
        gbd[ONES_ROW, r:r + HH] = gcp
        gobd[r:r + HH, 8 * c:8 * c + 8] = Go
        gobd[ONES_ROW, 8 * c:8 * c + 8] = gco

    bzm = np.zeros((128, 4), f32)
    for c in range(NCH):
        r = PITCH * c
        bzm[r:r + HH, 0] = b0
        bzm[r:r + HH, 1] = b1
        bzm[r:r + HH, 2] = b2

    z2init = np.zeros((128, COLS), f32)
    z2init[ONES_ROW, :] = 1.0

    # initial state per core: p0 = h0@W0, o0 = h0@Wl + bl
    h0 = initial @ Wi + bi                                 # [B, 32]
    p0 = h0 @ W0                                           # [B, 15]
    o0 = h0 @ Wl + bl                                      # [B, 8]
    s0p_list, s0o_list = [], []
    for core in range(NCORES):
        sp = np.zeros((128, COLS), f32)
        so = np.zeros((64, COLS), f32)
        for c in range(NCH):
            rows = slice(core * BSH + c * COLS, core * BSH + (c + 1) * COLS)
            sp[PITCH * c:PITCH * c + HH, :] = p0[rows].T
            so[8 * c:8 * c + 8, :] = o0[rows].T
        s0p_list.append(sp)
        s0o_list.append(so)

    shared = {
        "w1bd": w1bd, "w2bd": w2bd, "gbd": gbd, "gbdo": gobd, "bz": bzm,
        "z2init": z2init, "ident": np.eye(128, dtype=f32),
    }
    return shared, s0p_list, s0o_list


def unshard(scr_list, t_total=T):
    """scratch [64, T*64] per core -> full output [B, T, OUT]."""
    outs = []
    for scr in scr_list:
        s = scr.reshape(NCH, 8, t_total, COLS)             # [c, o, t, n]
        outs.append(np.ascontiguousarray(s.transpose(0, 3, 2, 1))
                    .reshape(BSH, t_total, 8))
    return np.concatenate(outs, axis=0)


_CACHE = {}


def _get_program(t_total=T, tbuf=TBUF, widths=WIDTHS):
    key = (t_total, tbuf, widths)
    if key not in _CACHE:
        _CACHE[key] = build_program(t_total, tbuf, widths)
    return _CACHE[key]


def kernel(**inputs) -> np.ndarray:
    from concourse.bass_utils import run_bass_kernel_spmd

    shared, s0p_list, s0o_list = prep_inputs(**inputs)
    nc = _get_program()
    in_maps = [dict(shared, s0p=s0p_list[core], s0o=s0o_list[core])
               for core in range(NCORES)]
    res = run_bass_kernel_spmd(nc, in_maps, core_ids=list(range(NCORES)))
    scr_list = [res.results[core]["oscr"] for core in range(NCORES)]
    return unshard(scr_list)


# revision 5
# speedup vs baseline: 1.1741x; 1.0006x over previous
"""Trainium2 Bass kernel for a NeuralODE (forward-Euler scan over a tiny MLP).

Reference computation (per batch row x of `initial`):
    h0 = x @ Wi + bi                                  # [32]
    h_{t+1} = h_t + dt * f(h_t),  t = 0..T-2
    f(h) = tanh(tanh(tanh(h@W0+b0)@W1+b1)@W2+b2) @ W3 + b3
    out[t] = h_t @ Wl + bl                            # [8], t = 0..T-1

Device reformulation (exact in exact arithmetic): track the projected state
    p_t = W0^T h_t   (15-dim)     o_t = Wl^T h_t + bl   (8-dim = the output!)
since h_t only ever enters through W0 (layer 0) and Wl (readout):
    z  = tanh(p + b0); z = tanh(z@W1+b1); z = tanh(z@W2+b2)
    p += dt * (z @ (W3@W0) + b3@W0)
    o += dt * (z @ (W3@Wl) + b3@Wl)
The o-part of the state IS the output trajectory.

Total time is dominated by the serial per-step dependency cycle
    act0 -> mm1 -> act1 -> mm2 -> act2 -> mmG -> act0(next step)
whose latency is almost entirely fixed engine/semaphore pipeline latency
(ACT ~370ns SBUF access+ack, PE ~173ns, ~120ns sem hops), so the design
minimizes per-instruction column counts and keeps everything else off the
cycle:

Per-core layout (8 cores, batch-sharded 4096 -> 512 each):
  512 batch rows = 8 chunks of 64 (columns of every tile).
  Chunk k occupies partition rows 15k..15k+14 (rows 0..119); row 120 is a
  constant-one row (z2 only) feeding the folded b3-biases; rows 121..127
  unused (excluded from every matmul contraction).
  Weights are host-assembled 128x128 block-diagonal matrices at 15-row
  pitch; dt is folded into the G matrices so the p-update is pure PSUM
  accumulation by the PE (start=False), keeping the DVE off the cycle.
  3 column-streams (22/21/21 cols) interleave so the cycle runs with
  minimal per-instruction processing time while the ACT engine (~93%
  busy) still keeps up.
  PSUM banks (8 available): per stream a persistent p-accumulator pg
  [128,w] (seeded via identity matmul so the PE sets the has_written
  bits) and one bank shared by p1/p2 (the WAR is covered by the z1 RAW
  edge); plus a single shared podelta bank [64,64] that each stream's
  G_o matmul writes as a fresh start/stop group each step.
  The o-trajectory is accumulated OFF the cycle by the DVE:
      blk[slot t] = blk[slot t-1] + podelta_t
  chaining through a [64, tbuf*w] ring (2-deep) whose complete blocks
  DMA to DRAM scratch [64, T*64] = (chunk,o; t,n); the host transposes
  scratch to out[c*64+n, t, o].

A pre-compile pass drops semaphore waits that are trivially satisfied by
same-engine program order; the surviving (cross-engine) wait then attaches
to the consuming instruction itself instead of a standalone EventSemaphore,
which would serialize the SEQ-side decode into the dependency cycle.
"""

from collections import defaultdict
from contextlib import ExitStack

import numpy as np

B, T = 4096, 1000
INIT_DIM, HID, HH, OUT = 16, 32, 15, 8
NCORES = 8
BSH = B // NCORES          # 512 batch rows per core
NCH = 8                    # chunks per core (64 batch cols each)
COLS = BSH // NCH          # 64
PITCH = 15                 # chunk partition pitch
ONES_ROW = 120             # z2 constant-one row
ACT_HI = 120               # activations write partitions [0, 120)
TBUF = 8                   # time slots per ring block (divides 1000)
WIDTHS = (22, 21, 21)      # column split across streams

_SYNC_OK = {
    "InstActivation", "InstMatmult", "InstTensorCopy", "InstMemset",
    "InstEventSemaphore", "InstTensorTensor", "InstTensorScalarPtr",
    "InstLdweights", "InstNoOp", "InstTensorReduce", "InstTensorScalar",
}


def strip_redundant_self_waits(nc):
    """Drop sem waits trivially satisfied by same-engine program order.

    A wait (S >= v) on engine-E instruction X is droppable iff every update
    to S module-wide is a plain `sem-inc` from a synchronous (non-DMA)
    instruction on engine E, and the cumulative update value from
    E-instructions preceding X in the same basic block is >= v.  Dropping
    the redundant wait lets the remaining cross-engine wait attach to X
    itself (TRN2 allows one attached wait per instruction), so X
    pre-decodes and fires as soon as the producer's semaphore arrives.
    """
    fn = nc.m.functions[0]
    sem_updaters = defaultdict(list)
    for b in fn.blocks:
        for inst in b.instructions:
            si = inst.sync_info
            if si is not None and si.on_update:
                for u in si.on_update:
                    sem_updaters[u.ant_name].append(
                        (inst.engine, type(inst).__name__, u.update_mode))

    def droppable_sem(name, engine):
        ups = sem_updaters.get(name)
        if not ups:
            return False
        return all(e == engine and t in _SYNC_OK and m == "sem-inc"
                   for (e, t, m) in ups)

    for b in fn.blocks:
        cum = defaultdict(int)
        for inst in b.instructions:
            si = inst.sync_info
            if si is not None and si.on_wait:
                keep = [w for w in si.on_wait if not (
                    w.sync_type == "semaphore"
                    and w.wait_mode == "sem-ge-imm"
                    and droppable_sem(w.ant_name, inst.engine)
                    and cum[(inst.engine, w.ant_name)] >= w.wait_value)]
                if len(keep) != len(si.on_wait):
                    si.on_wait = keep
            if si is not None and si.on_update:
                for u in si.on_update:
                    if u.update_mode == "sem-inc":
                        cum[(inst.engine, u.ant_name)] += u.update_value


def build_program(t_total=T, tbuf=TBUF, widths=WIDTHS):
    import concourse.tile as tile
    from concourse import bacc, mybir

    F32 = mybir.dt.float32
    Tanh = mybir.ActivationFunctionType.Tanh

    nc = bacc.Bacc("TRN2", target_bir_lowering=False, debug=False)

    s0p = nc.dram_tensor("s0p", [128, COLS], F32, kind="ExternalInput")
    s0o = nc.dram_tensor("s0o", [64, COLS], F32, kind="ExternalInput")
    w1 = nc.dram_tensor("w1bd", [128, 128], F32, kind="ExternalInput")
    w2 = nc.dram_tensor("w2bd", [128, 128], F32, kind="ExternalInput")
    gm = nc.dram_tensor("gbd", [128, 128], F32, kind="ExternalInput")
    gmo = nc.dram_tensor("gbdo", [128, 64], F32, kind="ExternalInput")
    bz = nc.dram_tensor("bz", [128, 4], F32, kind="ExternalInput")
    z2i = nc.dram_tensor("z2init", [128, COLS], F32, kind="ExternalInput")
    ident = nc.dram_tensor("ident", [128, 128], F32, kind="ExternalInput")
    scr = nc.dram_tensor("oscr", [64, t_total * COLS], F32,
                         kind="ExternalOutput")

    nb = t_total // tbuf
    assert nb * tbuf == t_total
    assert sum(widths) == COLS
    nstream = len(widths)

    with tile.TileContext(nc) as tc, ExitStack() as ctx:
        const = ctx.enter_context(tc.tile_pool(name="const", bufs=1))
        rings = [ctx.enter_context(tc.tile_pool(name=f"ring{s}", bufs=2))
                 for s in range(nstream)]
        psum = ctx.enter_context(tc.tile_pool(name="psum", bufs=1,
                                              space="PSUM"))

        # warm the tanh activation table immediately (zeroed scratch via the
        # otherwise-idle Pool engine) so the implicit table load (~1.3us)
        # runs during the constant DMAs instead of blocking the first act
        warm = const.tile([1, 1], F32, tag="warm")
        nc.gpsimd.memset(warm[:], 0.0)
        nc.scalar.activation(warm[:], warm[:], Tanh)

        # startup-critical tensors first so the scan starts while the
        # remaining weights stream in
        id_sb = const.tile([128, 128], F32, tag="ident")
        s0p_sb = const.tile([128, COLS], F32, tag="s0p")
        s0o_sb = const.tile([64, COLS], F32, tag="s0o")
        bz_sb = const.tile([128, 4], F32, tag="bz")
        z2_sb = const.tile([128, COLS], F32, tag="z2")
        w1_sb = const.tile([128, 128], F32, tag="w1")
        w2_sb = const.tile([128, 128], F32, tag="w2")
        g_sb = const.tile([128, 128], F32, tag="g")
        go_sb = const.tile([128, 64], F32, tag="go")
        nc.sync.dma_start(id_sb[:], ident.ap())
        nc.sync.dma_start(s0p_sb[:], s0p.ap())
        nc.sync.dma_start(bz_sb[:], bz.ap())
        nc.sync.dma_start(w1_sb[:], w1.ap())
        nc.sync.dma_start(z2_sb[:], z2i.ap())
        nc.sync.dma_start(w2_sb[:], w2.ap())
        nc.sync.dma_start(g_sb[:], gm.ap())
        nc.sync.dma_start(go_sb[:], gmo.ap())
        nc.sync.dma_start(s0o_sb[:], s0o.ap())

        z0_sb = const.tile([128, COLS], F32, tag="z0")
        z1_sb = const.tile([128, COLS], F32, tag="z1")
        podelta = psum.tile([64, COLS], F32, tag="podelta")

        class Stream:
            pass

        streams = []
        for s in range(nstream):
            st = Stream()
            st.lo = sum(widths[:s])
            st.w = widths[s]
            sl = slice(st.lo, st.lo + st.w)
            st.z0 = z0_sb[:, sl]
            st.z1 = z1_sb[:, sl]
            st.z2 = z2_sb[:, sl]
            st.p1 = psum.tile([128, st.w], F32, tag=f"p12_{s}",
                              name=f"p12_{s}")[:]
            st.p2 = st.p1
            st.pg = psum.tile([128, st.w], F32, tag=f"pg_{s}",
                              name=f"pg_{s}")[:]
            st.pd = podelta[:, sl]
            # seed the p accumulator via the PE (sets PSUM has_written bits)
            nc.tensor.matmul(st.pg, id_sb[:], s0p_sb[:, sl],
                             start=True, stop=False, skip_group_check=True)
            st.blks = {}
            streams.append(st)

        def get_blk(st, s_idx, k):
            if k not in st.blks:
                st.blks[k] = rings[s_idx].tile(
                    [64, tbuf * st.w], F32, tag=f"blk{s_idx}",
                    name=f"blk{s_idx}_{k}")
            return st.blks[k]

        def drain_o(st, blk, k):
            nc.sync.dma_start(
                scr.ap().rearrange("p (t n) -> p t n", n=COLS)[
                    :, k * tbuf:(k + 1) * tbuf, st.lo:st.lo + st.w],
                blk[:, :].rearrange("p (t n) -> p t n", n=st.w),
            )

        K1 = 120   # contraction rows for W1/W2 matmuls
        KG = 121   # contraction rows for G matmuls (incl ones-row)

        # ring slot 0 <- o_0 (initial readout)
        for s_idx, st in enumerate(streams):
            blk0 = get_blk(st, s_idx, 0)
            nc.vector.tensor_copy(blk0[:, 0:st.w],
                                  s0o_sb[:, st.lo:st.lo + st.w])

        for slot in range(1, t_total):
            k = slot // tbuf
            k1, i1 = divmod(slot - 1, tbuf)
            for s_idx, st in enumerate(streams):
                get_blk(st, s_idx, k1)
                get_blk(st, s_idx, k)
            for st in streams:
                nc.scalar.activation(st.z0[0:ACT_HI, :], st.pg[0:ACT_HI, :],
                                     Tanh, bias=bz_sb[0:ACT_HI, 0:1])
            for st in streams:
                nc.tensor.matmul(st.p1, w1_sb[0:K1, :], st.z0[0:K1, :],
                                 start=True, stop=True)
            for st in streams:
                nc.scalar.activation(st.z1[0:ACT_HI, :], st.p1[0:ACT_HI, :],
                                     Tanh, bias=bz_sb[0:ACT_HI, 1:2])
            for st in streams:
                nc.tensor.matmul(st.p2, w2_sb[0:K1, :], st.z1[0:K1, :],
                                 start=True, stop=True)
            for st in streams:
                nc.scalar.activation(st.z2[0:ACT_HI, :], st.p2[0:ACT_HI, :],
                                     Tanh, bias=bz_sb[0:ACT_HI, 2:3])
            for st in streams:
                nc.tensor.matmul(st.pg, g_sb[0:KG, :], st.z2[0:KG, :],
                                 start=False, stop=False,
                                 skip_group_check=True)
            for st in streams:
                nc.tensor.matmul(st.pd, go_sb[0:KG, :], st.z2[0:KG, :],
                                 start=True, stop=True,
                                 skip_group_check=True)
            # o_t = o_{t-1} + podelta, chained through the ring by the DVE
            ks, isl = divmod(slot, tbuf)
            for st in streams:
                prev = st.blks[k1][:, i1 * st.w:(i1 + 1) * st.w]
                cur = st.blks[ks][:, isl * st.w:(isl + 1) * st.w]
                nc.vector.tensor_add(cur, prev, st.pd)
            if slot % tbuf == 0:
                for st in streams:
                    drain_o(st, st.blks[k - 1], k - 1)

        kl = (t_total - 1) // tbuf
        for st in streams:
            drain_o(st, st.blks[kl], kl)

    strip_redundant_self_waits(nc)
    nc.compile()
    return nc


def prep_inputs(times, initial, Wi, bi, Wf0, bf0, Wf1, bf1, Wf2, bf2, Wf3, bf3,
                Wl, bl, t_total=T):
    """Host-side prep. Returns (shared input map, per-core s0p/s0o lists)."""
    f32 = np.float32
    times = np.asarray(times, f32)
    initial = np.asarray(initial, f32)
    Wi, bi = np.asarray(Wi, f32), np.asarray(bi, f32)
    W0, b0 = np.asarray(Wf0, f32), np.asarray(bf0, f32)
    W1, b1 = np.asarray(Wf1, f32), np.asarray(bf1, f32)
    W2, b2 = np.asarray(Wf2, f32), np.asarray(bf2, f32)
    W3, b3 = np.asarray(Wf3, f32), np.asarray(bf3, f32)
    Wl, bl = np.asarray(Wl, f32), np.asarray(bl, f32)

    dt = times[1:t_total] - times[:t_total - 1]
    assert np.all(dt == dt[0]), "kernel requires a constant time step"
    dt0 = float(dt[0])

    w1bd = np.zeros((128, 128), f32)
    w2bd = np.zeros((128, 128), f32)
    gbd = np.zeros((128, 128), f32)
    gobd = np.zeros((128, 64), f32)
    Gp = (W3 @ W0) * dt0                                   # [15, 15]
    Go = (W3 @ Wl) * dt0                                   # [15, 8]
    gcp = (b3 @ W0) * dt0                                  # [15]
    gco = (b3 @ Wl) * dt0                                  # [8]
    for c in range(NCH):
        r = PITCH * c
        w1bd[r:r + HH, r:r + HH] = W1
        w2bd[r:r + HH, r:r + HH] = W2
        gbd[r:r + HH, r:r + HH] = Gp
        gbd[ONES_ROW, r:r + HH] = gcp
        gobd[r:r + HH, 8 * c:8 * c + 8] = Go
        gobd[ONES_ROW, 8 * c:8 * c + 8] = gco

    bzm = np.zeros((128, 4), f32)
    for c in range(NCH):
        r = PITCH * c
        bzm[r:r + HH, 0] = b0
        bzm[r:r + HH, 1] = b1
        bzm[r:r + HH, 2] = b2

    z2init = np.zeros((128, COLS), f32)
    z2init[ONES_ROW, :] = 1.0

    # initial state per core: p0 = h0@W0, o0 = h0@Wl + bl
    h0 = initial @ Wi + bi                                 # [B, 32]
    p0 = h0 @ W0                                           # [B, 15]
    o0 = h0 @ Wl + bl                                      # [B, 8]
    s0p_list, s0o_list = [], []
    for core in range(NCORES):
        sp = np.zeros((128, COLS), f32)
        so = np.zeros((64, COLS), f32)
        for c in range(NCH):
            rows = slice(core * BSH + c * COLS, core * BSH + (c + 1) * COLS)
            sp[PITCH * c:PITCH * c + HH, :] = p0[rows].T
            so[8 * c:8 * c + 8, :] = o0[rows].T
        s0p_list.append(sp)
        s0o_list.append(so)

    shared = {
        "w1bd": w1bd, "w2bd": w2bd, "gbd": gbd, "gbdo": gobd, "bz": bzm,
        "z2init": z2init, "ident": np.eye(128, dtype=f32),
    }
    return shared, s0p_list, s0o_list


def unshard(scr_list, t_total=T):
    """scratch [64, T*64] per core -> full output [B, T, OUT]."""
    outs = []
    for scr in scr_list:
        s = scr.reshape(NCH, 8, t_total, COLS)             # [c, o, t, n]
        outs.append(np.ascontiguousarray(s.transpose(0, 3, 2, 1))
                    .reshape(BSH, t_total, 8))
    return np.concatenate(outs, axis=0)


_CACHE = {}


def _get_program(t_total=T, tbuf=TBUF, widths=WIDTHS):
    key = (t_total, tbuf, widths)
    if key not in _CACHE:
        _CACHE[key] = build_program(t_total, tbuf, widths)
    return _CACHE[key]


def kernel(**inputs) -> np.ndarray:
    from concourse.bass_utils import run_bass_kernel_spmd

    shared, s0p_list, s0o_list = prep_inputs(**inputs)
    nc = _get_program()
    in_maps = [dict(shared, s0p=s0p_list[core], s0o=s0o_list[core])
               for core in range(NCORES)]
    res = run_bass_kernel_spmd(nc, in_maps, core_ids=list(range(NCORES)))
    scr_list = [res.results[core]["oscr"] for core in range(NCORES)]
    return unshard(scr_list)


# revision 18
# speedup vs baseline: 1.1760x; 1.0016x over previous
"""Trainium2 Bass kernel for a NeuralODE (forward-Euler scan over a tiny MLP).

Reference computation (per batch row x of `initial`):
    h0 = x @ Wi + bi                                  # [32]
    h_{t+1} = h_t + dt * f(h_t),  t = 0..T-2
    f(h) = tanh(tanh(tanh(h@W0+b0)@W1+b1)@W2+b2) @ W3 + b3
    out[t] = h_t @ Wl + bl                            # [8], t = 0..T-1

Device reformulation (exact in exact arithmetic): track the projected state
    p_t = W0^T h_t   (15-dim)     o_t = Wl^T h_t + bl   (8-dim = the output!)
since h_t only ever enters through W0 (layer 0) and Wl (readout):
    z  = tanh(p + b0); z = tanh(z@W1+b1); z = tanh(z@W2+b2)
    p += dt * (z @ (W3@W0) + b3@W0)
    o += dt * (z @ (W3@Wl) + b3@Wl)
The o-part of the state IS the output trajectory.

Total time is dominated by the serial per-step dependency cycle
    act0 -> mm1 -> act1 -> mm2 -> act2 -> mmG -> act0(next step)
whose latency is almost entirely fixed engine/semaphore pipeline latency
(ACT ~370ns SBUF access+ack, PE ~173ns, ~120ns sem hops), so the design
minimizes per-instruction column counts and keeps everything else off the
cycle:

Per-core layout (8 cores, batch-sharded 4096 -> 512 each):
  512 batch rows = 8 chunks of 64 (columns of every tile).
  Chunk k occupies partition rows 15k..15k+14 (rows 0..119); row 120 is a
  constant-one row (z2 only) feeding the folded b3-biases; rows 121..127
  unused (excluded from every matmul contraction).
  Weights are host-assembled 128x128 block-diagonal matrices at 15-row
  pitch; dt is folded into the G matrices so the p-update is pure PSUM
  accumulation by the PE (start=False), keeping the DVE off the cycle.
  3 column-streams (22/21/21 cols) interleave so the cycle runs with
  minimal per-instruction processing time while the ACT engine (~93%
  busy) still keeps up.
  PSUM banks (8 available): per stream a persistent p-accumulator pg
  [128,w] (seeded via identity matmul so the PE sets the has_written
  bits) and one bank shared by p1/p2 (the WAR is covered by the z1 RAW
  edge); plus a single shared podelta bank [64,64] that each stream's
  G_o matmul writes as a fresh start/stop group each step.
  The o-trajectory is accumulated OFF the cycle by the DVE:
      blk[slot t] = blk[slot t-1] + podelta_t
  chaining through a [64, tbuf*w] ring (2-deep) whose complete blocks
  DMA to DRAM scratch [64, T*64] = (chunk,o; t,n); the host transposes
  scratch to out[c*64+n, t, o].

A pre-compile pass drops semaphore waits that are trivially satisfied by
same-engine program order; the surviving (cross-engine) wait then attaches
to the consuming instruction itself instead of a standalone EventSemaphore,
which would serialize the SEQ-side decode into the dependency cycle.
"""

from collections import defaultdict
from contextlib import ExitStack

import numpy as np

B, T = 4096, 1000
INIT_DIM, HID, HH, OUT = 16, 32, 15, 8
NCORES = 8
BSH = B // NCORES          # 512 batch rows per core
NCH = 8                    # chunks per core (64 batch cols each)
COLS = BSH // NCH          # 64
PITCH = 15                 # chunk partition pitch
ONES_ROW = 120             # z2 constant-one row
ACT_HI = 120               # activations write partitions [0, 120)
TBUF = 8                   # time slots per ring block (divides 1000)
WIDTHS = (22, 21, 21)      # column split across streams

_SYNC_OK = {
    "InstActivation", "InstMatmult", "InstTensorCopy", "InstMemset",
    "InstEventSemaphore", "InstTensorTensor", "InstTensorScalarPtr",
    "InstLdweights", "InstNoOp", "InstTensorReduce", "InstTensorScalar",
}


def strip_redundant_self_waits(nc):
    """Drop sem waits trivially satisfied by same-engine program order.

    A wait (S >= v) on engine-E instruction X is droppable iff every update
    to S module-wide is a plain `sem-inc` from a synchronous (non-DMA)
    instruction on engine E, and the cumulative update value from
    E-instructions preceding X in the same basic block is >= v.  Dropping
    the redundant wait lets the remaining cross-engine wait attach to X
    itself (TRN2 allows one attached wait per instruction), so X
    pre-decodes and fires as soon as the producer's semaphore arrives.
    """
    fn = nc.m.functions[0]
    sem_updaters = defaultdict(list)
    for b in fn.blocks:
        for inst in b.instructions:
            si = inst.sync_info
            if si is not None and si.on_update:
                for u in si.on_update:
                    sem_updaters[u.ant_name].append(
                        (inst.engine, type(inst).__name__, u.update_mode))

    def droppable_sem(name, engine):
        ups = sem_updaters.get(name)
        if not ups:
            return False
        return all(e == engine and t in _SYNC_OK and m == "sem-inc"
                   for (e, t, m) in ups)

    for b in fn.blocks:
        cum = defaultdict(int)
        for inst in b.instructions:
            si = inst.sync_info
            if si is not None and si.on_wait:
                keep = [w for w in si.on_wait if not (
                    w.sync_type == "semaphore"
                    and w.wait_mode == "sem-ge-imm"
                    and droppable_sem(w.ant_name, inst.engine)
                    and cum[(inst.engine, w.ant_name)] >= w.wait_value)]
                if len(keep) != len(si.on_wait):
                    si.on_wait = keep
            if si is not None and si.on_update:
                for u in si.on_update:
                    if u.update_mode == "sem-inc":
                        cum[(inst.engine, u.ant_name)] += u.update_value


def build_program(t_total=T, tbuf=TBUF, widths=WIDTHS):
    import concourse.tile as tile
    from concourse import bacc, mybir

    F32 = mybir.dt.float32
    Tanh = mybir.ActivationFunctionType.Tanh

    nc = bacc.Bacc("TRN2", target_bir_lowering=False, debug=False)

    # cst1 = [ident | s0p | bz | w1] (startup-critical), cst2 = [z2i | w2 |
    # g | go]: two packed DMAs instead of nine serialized HWDGE issues
    cst1 = nc.dram_tensor("cst1", [128, 324], F32, kind="ExternalInput")
    cst2 = nc.dram_tensor("cst2", [128, 384], F32, kind="ExternalInput")
    s0o = nc.dram_tensor("s0o", [64, COLS], F32, kind="ExternalInput")
    scr = nc.dram_tensor("oscr", [64, t_total * COLS], F32,
                         kind="ExternalOutput")

    nb = t_total // tbuf
    assert nb * tbuf == t_total
    assert sum(widths) == COLS
    nstream = len(widths)

    with tile.TileContext(nc) as tc, ExitStack() as ctx:
        const = ctx.enter_context(tc.tile_pool(name="const", bufs=1))
        ring = ctx.enter_context(tc.tile_pool(name="ring", bufs=2))
        psum = ctx.enter_context(tc.tile_pool(name="psum", bufs=1,
                                              space="PSUM"))

        # warm the tanh activation table immediately (zeroed scratch via the
        # otherwise-idle Pool engine) so the implicit table load (~1.3us)
        # runs during the constant DMAs instead of blocking the first act
        warm = const.tile([1, 1], F32, tag="warm")
        nc.gpsimd.memset(warm[:], 0.0)
        nc.scalar.activation(warm[:], warm[:], Tanh)

        # startup-critical tensors first so the scan starts while the
        # remaining weights stream in
        cst1_sb = const.tile([128, 324], F32, tag="cst1")
        cst2_sb = const.tile([128, 384], F32, tag="cst2")
        s0o_sb = const.tile([64, COLS], F32, tag="s0o")
        id_sb = cst1_sb[:, 0:128]
        s0p_sb = cst1_sb[:, 128:128 + COLS]
        bz_sb = cst1_sb[:, 192:196]
        w1_sb = cst1_sb[:, 196:324]
        z2_sb = cst2_sb[:, 0:COLS]
        w2_sb = cst2_sb[:, 64:192]
        g_sb = cst2_sb[:, 192:320]
        go_sb = cst2_sb[:, 320:384]
        nc.sync.dma_start(cst1_sb[:], cst1.ap())
        nc.sync.dma_start(cst2_sb[:], cst2.ap())
        nc.sync.dma_start(s0o_sb[:], s0o.ap())

        z0_sb = const.tile([128, COLS], F32, tag="z0")
        z1_sb = const.tile([128, COLS], F32, tag="z1")
        podelta = psum.tile([64, COLS], F32, tag="podelta")

        class Stream:
            pass

        streams = []
        for s in range(nstream):
            st = Stream()
            st.lo = sum(widths[:s])
            st.w = widths[s]
            sl = slice(st.lo, st.lo + st.w)
            st.z0 = z0_sb[:, sl]
            st.z1 = z1_sb[:, sl]
            st.z2 = z2_sb[:, sl]
            st.p1 = psum.tile([128, st.w], F32, tag=f"p12_{s}",
                              name=f"p12_{s}")[:]
            st.p2 = st.p1
            st.pg = psum.tile([128, st.w], F32, tag=f"pg_{s}",
                              name=f"pg_{s}")[:]
            st.pd = podelta[:, sl]
            # seed the p accumulator via the PE (sets PSUM has_written bits)
            nc.tensor.matmul(st.pg, id_sb[:], s0p_sb[:, sl],
                             start=True, stop=False, skip_group_check=True)
            streams.append(st)

        # one ring block shared by all streams (disjoint column slices) so
        # each complete block drains with a single DMA
        blks = {}

        def get_blk(k):
            if k not in blks:
                blks[k] = ring.tile([64, tbuf * COLS], F32, tag="blk",
                                    name=f"blk_{k}")
            return blks[k]

        def oslice(blk, i, st):
            return blk[:, i * COLS + st.lo:i * COLS + st.lo + st.w]

        def drain_o(blk, k):
            nc.sync.dma_start(
                scr.ap().rearrange("p (t n) -> p t n", n=COLS)[
                    :, k * tbuf:(k + 1) * tbuf, :],
                blk[:, :].rearrange("p (t n) -> p t n", n=COLS),
            )

        K1 = 120   # contraction rows for W1/W2 matmuls
        KG = 121   # contraction rows for G matmuls (incl ones-row)

        # ring slot 0 <- o_0 (initial readout)
        blk0 = get_blk(0)
        for st in streams:
            nc.vector.tensor_copy(oslice(blk0, 0, st),
                                  s0o_sb[:, st.lo:st.lo + st.w])

        for slot in range(1, t_total):
            k = slot // tbuf
            k1, i1 = divmod(slot - 1, tbuf)
            get_blk(k1)
            get_blk(k)
            for st in streams:
                nc.scalar.activation(st.z0[0:ACT_HI, :], st.pg[0:ACT_HI, :],
                                     Tanh, bias=bz_sb[0:ACT_HI, 0:1])
            for st in streams:
                nc.tensor.matmul(st.p1, w1_sb[0:K1, :], st.z0[0:K1, :],
                                 start=True, stop=True)
            for st in streams:
                nc.scalar.activation(st.z1[0:ACT_HI, :], st.p1[0:ACT_HI, :],
                                     Tanh, bias=bz_sb[0:ACT_HI, 1:2])
            for st in streams:
                nc.tensor.matmul(st.p2, w2_sb[0:K1, :], st.z1[0:K1, :],
                                 start=True, stop=True)
            for st in streams:
                nc.scalar.activation(st.z2[0:ACT_HI, :], st.p2[0:ACT_HI, :],
                                     Tanh, bias=bz_sb[0:ACT_HI, 2:3])
            if slot < t_total - 1:   # p_{T} is never read
                for st in streams:
                    nc.tensor.matmul(st.pg, g_sb[0:KG, :], st.z2[0:KG, :],
                                     start=False, stop=False,
                                     skip_group_check=True)
            for st in streams:
                nc.tensor.matmul(st.pd, go_sb[0:KG, :], st.z2[0:KG, :],
                                 start=True, stop=True,
                                 skip_group_check=True)
            # o_t = o_{t-1} + podelta, chained through the ring by the DVE
            ks, isl = divmod(slot, tbuf)
            if slot == t_total - 1:
                # final slot: one full-width add (nothing downstream to
                # decouple, and it shortens the serial tail)
                nc.vector.tensor_add(
                    blks[ks][:, isl * COLS:(isl + 1) * COLS],
                    blks[k1][:, i1 * COLS:(i1 + 1) * COLS], podelta[:])
            else:
                for st in streams:
                    nc.vector.tensor_add(oslice(blks[ks], isl, st),
                                         oslice(blks[k1], i1, st), st.pd)
            if slot % tbuf == 0:
                drain_o(blks[k - 1], k - 1)

        # split the final block's drain: slots [kl*tbuf, T-1) can transfer
        # while the last slot's DVE add still runs; the tail then waits only
        # on a single-slot DMA
        kl = (t_total - 1) // tbuf
        il = (t_total - 1) % tbuf
        scr_tn = scr.ap().rearrange("p (t n) -> p t n", n=COLS)
        blk_tn = blks[kl][:, :].rearrange("p (t n) -> p t n", n=COLS)
        if il > 0:
            nc.sync.dma_start(
                scr_tn[:, kl * tbuf:kl * tbuf + il, :], blk_tn[:, 0:il, :])
        nc.sync.dma_start(
            scr_tn[:, t_total - 1:t_total, :], blk_tn[:, il:il + 1, :])

    strip_redundant_self_waits(nc)
    nc.compile()
    return nc


def prep_inputs(times, initial, Wi, bi, Wf0, bf0, Wf1, bf1, Wf2, bf2, Wf3, bf3,
                Wl, bl, t_total=T):
    """Host-side prep. Returns (shared input map, per-core s0p/s0o lists)."""
    f32 = np.float32
    times = np.asarray(times, f32)
    initial = np.asarray(initial, f32)
    Wi, bi = np.asarray(Wi, f32), np.asarray(bi, f32)
    W0, b0 = np.asarray(Wf0, f32), np.asarray(bf0, f32)
    W1, b1 = np.asarray(Wf1, f32), np.asarray(bf1, f32)
    W2, b2 = np.asarray(Wf2, f32), np.asarray(bf2, f32)
    W3, b3 = np.asarray(Wf3, f32), np.asarray(bf3, f32)
    Wl, bl = np.asarray(Wl, f32), np.asarray(bl, f32)

    dt = times[1:t_total] - times[:t_total - 1]
    assert np.all(dt == dt[0]), "kernel requires a constant time step"
    dt0 = float(dt[0])

    w1bd = np.zeros((128, 128), f32)
    w2bd = np.zeros((128, 128), f32)
    gbd = np.zeros((128, 128), f32)
    gobd = np.zeros((128, 64), f32)
    Gp = (W3 @ W0) * dt0                                   # [15, 15]
    Go = (W3 @ Wl) * dt0                                   # [15, 8]
    gcp = (b3 @ W0) * dt0                                  # [15]
    gco = (b3 @ Wl) * dt0                                  # [8]
    for c in range(NCH):
        r = PITCH * c
        w1bd[r:r + HH, r:r + HH] = W1
        w2bd[r:r + HH, r:r + HH] = W2
        gbd[r:r + HH, r:r + HH] = Gp
        gbd[ONES_ROW, r:r + HH] = gcp
        gobd[r:r + HH, 8 * c:8 * c + 8] = Go
        gobd[ONES_ROW, 8 * c:8 * c + 8] = gco

    bzm = np.zeros((128, 4), f32)
    for c in range(NCH):
        r = PITCH * c
        bzm[r:r + HH, 0] = b0
        bzm[r:r + HH, 1] = b1
        bzm[r:r + HH, 2] = b2

    z2init = np.zeros((128, COLS), f32)
    z2init[ONES_ROW, :] = 1.0

    # initial state per core: p0 = h0@W0, o0 = h0@Wl + bl
    h0 = initial @ Wi + bi                                 # [B, 32]
    p0 = h0 @ W0                                           # [B, 15]
    o0 = h0 @ Wl + bl                                      # [B, 8]
    s0p_list, s0o_list = [], []
    for core in range(NCORES):
        sp = np.zeros((128, COLS), f32)
        so = np.zeros((64, COLS), f32)
        for c in range(NCH):
            rows = slice(core * BSH + c * COLS, core * BSH + (c + 1) * COLS)
            sp[PITCH * c:PITCH * c + HH, :] = p0[rows].T
            so[8 * c:8 * c + 8, :] = o0[rows].T
        s0p_list.append(sp)
        s0o_list.append(so)

    # pack the device constants: cst1 (per-core, startup-critical) and
    # cst2 (shared) each load with a single DMA
    cst2 = np.concatenate([z2init, w2bd, gbd, gobd], axis=1)   # [128, 384]
    eye = np.eye(128, dtype=f32)
    cst1_list = [np.concatenate([eye, sp, bzm, w1bd], axis=1)  # [128, 324]
                 for sp in s0p_list]
    shared = {"cst2": cst2}
    return shared, cst1_list, s0o_list


def unshard(scr_list, t_total=T):
    """scratch [64, T*64] per core -> full output [B, T, OUT]."""
    outs = []
    for scr in scr_list:
        s = scr.reshape(NCH, 8, t_total, COLS)             # [c, o, t, n]
        outs.append(np.ascontiguousarray(s.transpose(0, 3, 2, 1))
                    .reshape(BSH, t_total, 8))
    return np.concatenate(outs, axis=0)


_CACHE = {}


def _get_program(t_total=T, tbuf=TBUF, widths=WIDTHS):
    key = (t_total, tbuf, widths)
    if key not in _CACHE:
        _CACHE[key] = build_program(t_total, tbuf, widths)
    return _CACHE[key]


def kernel(**inputs) -> np.ndarray:
    from concourse.bass_utils import run_bass_kernel_spmd

    shared, cst1_list, s0o_list = prep_inputs(**inputs)
    nc = _get_program()
    in_maps = [dict(shared, cst1=cst1_list[core], s0o=s0o_list[core])
               for core in range(NCORES)]
    res = run_bass_kernel_spmd(nc, in_maps, core_ids=list(range(NCORES)))
    scr_list = [res.results[core]["oscr"] for core in range(NCORES)]
    return unshard(scr_list)
